# revision 9
# baseline (speedup 1.0000x reference)
"""Trainium2 Bass kernel for a stride-2 4x4 ConvTranspose2d with
per-kernel-position bias (bias added before the overlap-add fold).

Shapes (hardcoded):
  x:      (8, 256, 64, 64)  f32
  weight: (128, 256, 4, 4)  f32
  bias:   (128, 4, 4)       f32
  out:    (8, 128, 130, 130) f32   [nh = (64-1)*2+4 = 130]

Strategy: data-parallel over batch - one sample per NeuronCore, 8 cores.
Per core the deconv is computed as 4 output-phase planes (p%2, q%2), each
a 65x65 image. Each phase plane is the sum of 4 shifted matmuls (the
kernel positions sharing that parity) accumulated directly in PSUM:

  plane[o, P, Q] = sum_{a,b in {0,1}} W[:, :, py+2a, px+2b]^T @ xp[:, P-a+1, Q-b+1]

All matmul inputs are bf16 (rel err ~4e-3 of output scale, far under the
2e-2 gate); bf16 enables FWL so LDWEIGHTS fully overlaps streaming and
the PE runs at its ~2.4GHz roofline. Inputs are spread across four DMA
queues with explicit completion-ordering (add_dep_helper) so the first
chunk's data (xp rows 0-15 + phase-0 weights) is not round-robin-delayed
by the bulk. The per-kernel-position bias with edge corrections baked in
is 3 host-precomputed rows (top/interior/bottom) per phase; drains add
them via stride-0 broadcast APs, writing bf16 planes streamed out in 3
DMAs per phase. The host interleaves the 4 bf16 planes into the strided
f32 (130,130) output.
"""

import numpy as np

B, CI, H, W = 8, 256, 64, 64
CO, KH, KW = 128, 4, 4
NH = NW = 130
NP = 65          # phase plane side
PADH = 66        # padded x rows (+1 top, +1 bottom)
PADW = 68        # padded x cols (+1 left, +3 right; row stride 2*68 elems)
NCORES = 8

CHUNK_ROWS = [7, 7, 7, 7, 7, 7, 7, 7, 7, 2]
assert sum(CHUNK_ROWS) == NP
N_WARMUP = 8


def _build_nc():
    import concourse.mybir as mybir
    import concourse.tile as tile
    from concourse import bacc
    from concourse.tile_rust import add_dep_helper

    f32 = mybir.dt.float32
    bf16 = mybir.dt.bfloat16

    nc = bacc.Bacc(None)
    # xp layout: [i=128, row=66, kt=2, col=68] bf16 - kt inner so a row
    # range is one contiguous DMA descriptor.
    xp_ext = nc.declare_dram_parameter("xp", [128, PADH * 2 * PADW], bf16, isOutput=False)
    # wt layout: [i=128, phase=4, kt=2, tap=4, o=128] bf16 - phase outer so
    # phase-0 weights arrive in one small early DMA.
    wt_ext = nc.declare_dram_parameter("wt", [128, 4 * 2 * 4 * CO], bf16, isOutput=False)
    # bias rows: [o=128, phase=4, kind=3 (top/int/bottom), 65] f32
    bv_ext = nc.declare_dram_parameter("bv", [128, 4 * 3 * NP], f32, isOutput=False)
    out_ext = nc.declare_dram_parameter("out", [4, CO, NP * NP], bf16, isOutput=True)

    with tile.TileContext(nc) as tc:
        with (
            tc.tile_pool(name="const", bufs=1) as cpool,
            tc.tile_pool(name="psum", bufs=6, space="PSUM") as ppool,
        ):
            xp_t = cpool.tile([128, PADH, 2, PADW], bf16, tag="xp", name="xp")
            w_t = cpool.tile([128, 4, 2, 4, CO], bf16, tag="w", name="w")
            bv_t = cpool.tile([128, 4, 3, NP], f32, tag="bv", name="bv")
            planes = [
                cpool.tile([128, NP, NP], bf16, tag=f"plane{p}", name=f"plane{p}")
                for p in range(4)
            ]

            # Input DMAs on the three DMA-capable queues; the bulk is
            # completion-chained (add_dep_helper) behind the first-needed
            # pieces so round-robin doesn't delay the first chunk's data:
            #   sync:   xp rows 0-16 (chunks 0-1), later the out DMAs
            #   gpsimd: xp rows 16-43 free-running; rows 43-66 after g0
            #   scalar: w phase0 -> bias rows -> w phases 1-3 (chained)
            g0 = nc.sync.dma_start(xp_t[:, 0:16], xp_ext[:, 0:16 * 2 * PADW])
            w0 = nc.scalar.dma_start(w_t[:, 0], wt_ext[:, 0:2 * 4 * CO])
            bv = nc.scalar.dma_start(bv_t[:], bv_ext[:])
            w123 = nc.scalar.dma_start(w_t[:, 1:4], wt_ext[:, 2 * 4 * CO:])
            mid = nc.gpsimd.dma_start(
                xp_t[:, 16:43], xp_ext[:, 16 * 2 * PADW:43 * 2 * PADW]
            )
            hi = nc.gpsimd.dma_start(
                xp_t[:, 43:66], xp_ext[:, 43 * 2 * PADW:]
            )
            add_dep_helper(g0.ins, hi.ins, sync=True, reason="xp-hi after xp-g0")
            add_dep_helper(w0.ins, bv.ins, sync=True, reason="bias after w0")
            add_dep_helper(bv.ins, w123.ins, sync=True, reason="w123 after bias")

            # PE warm-up: dummy bf16 matmuls with no input deps run during
            # the input-load window so HAM un-throttles (~5us of sustained
            # PE activity) by the time real matmuls begin.
            warm_in = cpool.tile([128, 512], bf16, tag="warm_in", name="warm_in")
            warm_ps = ppool.tile([128, 512], f32, tag="warm_ps", name="warm_ps", bufs=1)
            nc.gpsimd.memset(warm_in[:], 1.0)
            for _ in range(N_WARMUP):
                nc.tensor.matmul(
                    warm_ps[:], warm_in[:, 0:128], warm_in[:],
                    start=True, stop=True,
                )

            for pidx in range(4):
                pstart = 0
                for ci, pn in enumerate(CHUNK_ROWS):
                    nn = pn * NP
                    ps = ppool.tile([128, 512], f32, tag="acc", name=f"acc{pidx}_{ci}")
                    mm = 0
                    for a in (0, 1):
                        for b in (0, 1):
                            for kt in (0, 1):
                                lhsT = w_t[:, pidx, kt, a * 2 + b, :]
                                rhs = xp_t[
                                    :,
                                    pstart + 1 - a: pstart + 1 - a + pn,
                                    kt,
                                    1 - b: 1 - b + NP,
                                ]
                                nc.tensor.matmul(
                                    ps[:, 0:nn], lhsT, rhs,
                                    start=(mm == 0), stop=(mm == 7),
                                )
                                mm += 1
                    # drain: PSUM + bias rows -> bf16 plane.
                    # bias row kinds: 0=top (P=0), 1=interior, 2=bottom (P=64)
                    pl = planes[pidx]
                    if ci == 0:
                        nc.vector.tensor_add(
                            pl[:, 0:1, :],
                            ps[:, 0:NP].rearrange("p (a b) -> p a b", b=NP),
                            bv_t[:, pidx, 0:1, :],
                        )
                        nc.vector.tensor_add(
                            pl[:, 1:7, :],
                            ps[:, NP:7 * NP].rearrange("p (a b) -> p a b", b=NP),
                            bv_t[:, pidx, 1:2, :].broadcast_to([128, 6, NP]),
                        )
                    elif ci == 9:
                        nc.vector.tensor_add(
                            pl[:, 63:64, :],
                            ps[:, 0:NP].rearrange("p (a b) -> p a b", b=NP),
                            bv_t[:, pidx, 1:2, :],
                        )
                        nc.vector.tensor_add(
                            pl[:, 64:65, :],
                            ps[:, NP:2 * NP].rearrange("p (a b) -> p a b", b=NP),
                            bv_t[:, pidx, 2:3, :],
                        )
                    else:
                        nc.vector.tensor_add(
                            pl[:, pstart:pstart + pn, :],
                            ps[:, 0:nn].rearrange("p (a b) -> p a b", b=NP),
                            bv_t[:, pidx, 1:2, :].broadcast_to([128, pn, NP]),
                        )
                    pstart += pn
                    if ci == 4:
                        nc.sync.dma_start(
                            out_ext[pidx, :, 0:35 * NP],
                            planes[pidx][:, 0:35, :],
                        )
                    elif ci == 7:
                        nc.sync.dma_start(
                            out_ext[pidx, :, 35 * NP:56 * NP],
                            planes[pidx][:, 35:56, :],
                        )
                    elif ci == 9:
                        nc.sync.dma_start(
                            out_ext[pidx, :, 56 * NP:],
                            planes[pidx][:, 56:, :],
                        )
    nc.compile()
    return nc


def _host_prep(x, weight, bias):
    import ml_dtypes

    # padded, i-tiled x: [B, 128, row=66, kt=2, col=68] bf16
    xp = np.zeros((B, 128, PADH, 2, PADW), dtype=ml_dtypes.bfloat16)
    xr = x.reshape(B, 2, 128, H, W).transpose(0, 2, 3, 1, 4)  # b,i,h,kt,w
    xp[:, :, 1:65, :, 1:65] = xr.astype(ml_dtypes.bfloat16)
    xp = np.ascontiguousarray(xp.reshape(B, 128, PADH * 2 * PADW))

    # weights as lhsT: wt[i, phase, kt, tap=(a,b), o] = weight[o, kt*128+i, py+2a, px+2b]
    wr = weight.reshape(CO, 2, 128, 4, 4)
    wt = np.empty((128, 4, 2, 4, CO), dtype=ml_dtypes.bfloat16)
    for py in range(2):
        for px in range(2):
            for a in range(2):
                for bb in range(2):
                    wt[:, py * 2 + px, :, a * 2 + bb, :] = (
                        wr[:, :, :, py + 2 * a, px + 2 * bb]
                        .transpose(2, 1, 0).astype(ml_dtypes.bfloat16)
                    )
    wt = np.ascontiguousarray(wt).reshape(128, 4 * 2 * 4 * CO)

    # bias rows [o, phase, kind=3, 65] f32: kind 0 = P=0 (top), 1 = interior
    # P, 2 = P=64 (bottom); per-column validity baked in.
    bv = np.zeros((128, 4, 3, NP), dtype=np.float32)
    bias = bias.astype(np.float32)
    for py in range(2):
        for px in range(2):
            p = py * 2 + px
            for kind, avalid in ((0, (0,)), (1, (0, 1)), (2, (1,))):
                for q in range(NP):
                    s = np.zeros(128, dtype=np.float32)
                    for a in range(2):
                        if a not in avalid:
                            continue
                        for b2 in range(2):
                            if q == 0 and b2 == 1:
                                continue
                            if q == NP - 1 and b2 == 0:
                                continue
                            s += bias[:, py + 2 * a, px + 2 * b2]
                    bv[:, p, kind, q] = s
    bv = np.ascontiguousarray(bv.reshape(128, 4 * 3 * NP))
    return xp, wt, bv


_NC_CACHE = {}


def _get_nc():
    if "nc" not in _NC_CACHE:
        _NC_CACHE["nc"] = _build_nc()
    return _NC_CACHE["nc"]


def kernel(x, weight, bias, _trace=False, _trace_kwargs=None):
    from concourse.bass_utils import run_bass_kernel_spmd

    x = np.asarray(x, dtype=np.float32)
    weight = np.asarray(weight, dtype=np.float32)
    bias = np.asarray(bias, dtype=np.float32)
    xp, wt, bv = _host_prep(x, weight, bias)

    nc = _get_nc()
    in_maps = [{"xp": xp[b], "wt": wt, "bv": bv} for b in range(B)]
    res = run_bass_kernel_spmd(
        nc, in_maps, list(range(NCORES)),
        trace=_trace, **(_trace_kwargs or {}),
    )
    out = np.empty((B, CO, NH, NW), dtype=np.float32)
    for b in range(B):
        ph = res.results[b]["out"].reshape(4, CO, NP, NP).astype(np.float32)
        for py in range(2):
            for px in range(2):
                out[b, :, py::2, px::2] = ph[py * 2 + px]
    if _trace:
        kernel._last_results = res
    return out


# revision 11
# speedup vs baseline: 1.1170x; 1.1170x over previous
"""Trainium2 Bass kernel for a stride-2 4x4 ConvTranspose2d with
per-kernel-position bias (bias added before the overlap-add fold).

Shapes (hardcoded):
  x:      (8, 256, 64, 64)  f32
  weight: (128, 256, 4, 4)  f32
  bias:   (128, 4, 4)       f32
  out:    (8, 128, 130, 130) f32   [nh = (64-1)*2+4 = 130]

Strategy: data-parallel over batch - one sample per NeuronCore, 8 cores.
Per core the deconv is computed as 4 output-phase planes (p%2, q%2), each
a 65x65 image. Each phase plane is the sum of 4 shifted matmuls (the
kernel positions sharing that parity) accumulated directly in PSUM:

  plane[o, P, Q] = sum_{a,b in {0,1}} W[:, :, py+2a, px+2b]^T @ xp[:, P-a+1, Q-b+1]

All matmul inputs are bf16 (rel err ~4e-3 of output scale, far under the
2e-2 gate); bf16 enables FWL so LDWEIGHTS fully overlaps streaming and
the PE runs at its ~2.4GHz roofline. Inputs are spread across four DMA
queues with explicit completion-ordering (add_dep_helper) so the first
chunk's data (xp rows 0-15 + phase-0 weights) is not round-robin-delayed
by the bulk. The per-kernel-position bias with edge corrections baked in
is 3 host-precomputed rows (top/interior/bottom) per phase; drains add
them via stride-0 broadcast APs, writing bf16 planes streamed out in 3
DMAs per phase. The host interleaves the 4 bf16 planes into the strided
f32 (130,130) output.
"""

import numpy as np

B, CI, H, W = 8, 256, 64, 64
CO, KH, KW = 128, 4, 4
NH = NW = 130
NP = 65          # phase plane side
PADH = 66        # padded x rows (+1 top, +1 bottom)
PADW = 68        # padded x cols (+1 left, +3 right; row stride 2*68 elems)
NCORES = 8

CHUNK_ROWS = [7, 7, 7, 7, 7, 7, 7, 7, 7, 2]
assert sum(CHUNK_ROWS) == NP
N_WARMUP = 10


def _build_nc():
    import concourse.mybir as mybir
    import concourse.tile as tile
    from concourse import bacc
    from concourse.tile_rust import add_dep_helper

    f32 = mybir.dt.float32
    bf16 = mybir.dt.bfloat16

    nc = bacc.Bacc(None)
    # xp layout: [i=128, row=66, kt=2, col=68] bf16 - kt inner so a row
    # range is one contiguous DMA descriptor.
    xp_ext = nc.declare_dram_parameter("xp", [128, PADH * 2 * PADW], bf16, isOutput=False)
    # wt layout: [i=128, phase=4, kt=2, tap=4, o=128] bf16 - phase outer so
    # phase-0 weights arrive in one small early DMA.
    wt_ext = nc.declare_dram_parameter("wt", [128, 4 * 2 * 4 * CO], bf16, isOutput=False)
    # bias rows: [o=128, phase=4, kind=3 (top/int/bottom), 65] f32
    bv_ext = nc.declare_dram_parameter("bv", [128, 4 * 3 * NP], f32, isOutput=False)
    out_ext = nc.declare_dram_parameter("out", [4, CO, NP * NP], bf16, isOutput=True)

    with tile.TileContext(nc) as tc:
        with (
            tc.tile_pool(name="const", bufs=1) as cpool,
            tc.tile_pool(name="psum", bufs=6, space="PSUM") as ppool,
        ):
            xp_t = cpool.tile([128, PADH, 2, PADW], bf16, tag="xp", name="xp")
            w_t = cpool.tile([128, 4, 2, 4, CO], bf16, tag="w", name="w")
            bv_t = cpool.tile([128, 4, 3, NP], f32, tag="bv", name="bv")
            planes = [
                cpool.tile([128, NP, NP], bf16, tag=f"plane{p}", name=f"plane{p}")
                for p in range(4)
            ]

            # Input DMAs on the three DMA-capable queues. Completion-chaining
            # via add_dep_helper(later, earlier) - the LATER DMA's queue
            # instruction carries a wait on the EARLIER one's completion
            # semaphore - so in-queue round-robin can't starve the pieces the
            # first chunks need (xp rows 0-24 + phase-0 weights):
            #   sync:   xp rows 0-24 (chunks 0-3), later the out DMAs
            #   gpsimd: xp rows 24-43 free-running; rows 43-66 chained after
            #   scalar: w phase0 -> bias rows -> w phases 1-3 (chained)
            g0 = nc.sync.dma_start(xp_t[:, 0:24], xp_ext[:, 0:24 * 2 * PADW])
            w0 = nc.scalar.dma_start(w_t[:, 0], wt_ext[:, 0:2 * 4 * CO])
            bv = nc.scalar.dma_start(bv_t[:], bv_ext[:])
            w123 = nc.scalar.dma_start(w_t[:, 1:4], wt_ext[:, 2 * 4 * CO:])
            mid = nc.gpsimd.dma_start(
                xp_t[:, 24:43], xp_ext[:, 24 * 2 * PADW:43 * 2 * PADW]
            )
            hi = nc.gpsimd.dma_start(
                xp_t[:, 43:66], xp_ext[:, 43 * 2 * PADW:]
            )
            add_dep_helper(hi.ins, mid.ins, sync=True, reason="xp-hi waits xp-mid")
            add_dep_helper(bv.ins, w0.ins, sync=True, reason="bias waits w0")
            add_dep_helper(w123.ins, bv.ins, sync=True, reason="w123 waits bias")

            # PE warm-up: dummy bf16 matmuls with no input deps run during
            # the input-load window so HAM un-throttles (~5us of sustained
            # PE activity) by the time real matmuls begin. The memset goes on
            # the vector engine, which has no DMA queue and is never blocked.
            warm_in = cpool.tile([128, 512], bf16, tag="warm_in", name="warm_in")
            warm_ps = ppool.tile([128, 512], f32, tag="warm_ps", name="warm_ps", bufs=1)
            nc.vector.memset(warm_in[:], 1.0)
            for _ in range(N_WARMUP):
                nc.tensor.matmul(
                    warm_ps[:], warm_in[:, 0:128], warm_in[:],
                    start=True, stop=True,
                )

            for pidx in range(4):
                pstart = 0
                for ci, pn in enumerate(CHUNK_ROWS):
                    nn = pn * NP
                    ps = ppool.tile([128, 512], f32, tag="acc", name=f"acc{pidx}_{ci}")
                    mm = 0
                    for a in (0, 1):
                        for b in (0, 1):
                            for kt in (0, 1):
                                lhsT = w_t[:, pidx, kt, a * 2 + b, :]
                                rhs = xp_t[
                                    :,
                                    pstart + 1 - a: pstart + 1 - a + pn,
                                    kt,
                                    1 - b: 1 - b + NP,
                                ]
                                nc.tensor.matmul(
                                    ps[:, 0:nn], lhsT, rhs,
                                    start=(mm == 0), stop=(mm == 7),
                                )
                                mm += 1
                    # drain: PSUM + bias rows -> bf16 plane.
                    # bias row kinds: 0=top (P=0), 1=interior, 2=bottom (P=64)
                    pl = planes[pidx]
                    if ci == 0:
                        nc.vector.tensor_add(
                            pl[:, 0:1, :],
                            ps[:, 0:NP].rearrange("p (a b) -> p a b", b=NP),
                            bv_t[:, pidx, 0:1, :],
                        )
                        nc.vector.tensor_add(
                            pl[:, 1:7, :],
                            ps[:, NP:7 * NP].rearrange("p (a b) -> p a b", b=NP),
                            bv_t[:, pidx, 1:2, :].broadcast_to([128, 6, NP]),
                        )
                    elif ci == 9:
                        nc.vector.tensor_add(
                            pl[:, 63:64, :],
                            ps[:, 0:NP].rearrange("p (a b) -> p a b", b=NP),
                            bv_t[:, pidx, 1:2, :],
                        )
                        nc.vector.tensor_add(
                            pl[:, 64:65, :],
                            ps[:, NP:2 * NP].rearrange("p (a b) -> p a b", b=NP),
                            bv_t[:, pidx, 2:3, :],
                        )
                    else:
                        nc.vector.tensor_add(
                            pl[:, pstart:pstart + pn, :],
                            ps[:, 0:nn].rearrange("p (a b) -> p a b", b=NP),
                            bv_t[:, pidx, 1:2, :].broadcast_to([128, pn, NP]),
                        )
                    pstart += pn
                    if ci == 4:
                        nc.sync.dma_start(
                            out_ext[pidx, :, 0:35 * NP],
                            planes[pidx][:, 0:35, :],
                        )
                    elif ci == 7:
                        nc.sync.dma_start(
                            out_ext[pidx, :, 35 * NP:56 * NP],
                            planes[pidx][:, 35:56, :],
                        )
                    elif ci == 9:
                        nc.sync.dma_start(
                            out_ext[pidx, :, 56 * NP:],
                            planes[pidx][:, 56:, :],
                        )
    nc.compile()
    return nc


def _host_prep(x, weight, bias):
    import ml_dtypes

    # padded, i-tiled x: [B, 128, row=66, kt=2, col=68] bf16
    xp = np.zeros((B, 128, PADH, 2, PADW), dtype=ml_dtypes.bfloat16)
    xr = x.reshape(B, 2, 128, H, W).transpose(0, 2, 3, 1, 4)  # b,i,h,kt,w
    xp[:, :, 1:65, :, 1:65] = xr.astype(ml_dtypes.bfloat16)
    xp = np.ascontiguousarray(xp.reshape(B, 128, PADH * 2 * PADW))

    # weights as lhsT: wt[i, phase, kt, tap=(a,b), o] = weight[o, kt*128+i, py+2a, px+2b]
    wr = weight.reshape(CO, 2, 128, 4, 4)
    wt = np.empty((128, 4, 2, 4, CO), dtype=ml_dtypes.bfloat16)
    for py in range(2):
        for px in range(2):
            for a in range(2):
                for bb in range(2):
                    wt[:, py * 2 + px, :, a * 2 + bb, :] = (
                        wr[:, :, :, py + 2 * a, px + 2 * bb]
                        .transpose(2, 1, 0).astype(ml_dtypes.bfloat16)
                    )
    wt = np.ascontiguousarray(wt).reshape(128, 4 * 2 * 4 * CO)

    # bias rows [o, phase, kind=3, 65] f32: kind 0 = P=0 (top), 1 = interior
    # P, 2 = P=64 (bottom); per-column validity baked in.
    bv = np.zeros((128, 4, 3, NP), dtype=np.float32)
    bias = bias.astype(np.float32)
    for py in range(2):
        for px in range(2):
            p = py * 2 + px
            for kind, avalid in ((0, (0,)), (1, (0, 1)), (2, (1,))):
                for q in range(NP):
                    s = np.zeros(128, dtype=np.float32)
                    for a in range(2):
                        if a not in avalid:
                            continue
                        for b2 in range(2):
                            if q == 0 and b2 == 1:
                                continue
                            if q == NP - 1 and b2 == 0:
                                continue
                            s += bias[:, py + 2 * a, px + 2 * b2]
                    bv[:, p, kind, q] = s
    bv = np.ascontiguousarray(bv.reshape(128, 4 * 3 * NP))
    return xp, wt, bv


_NC_CACHE = {}


def _get_nc():
    if "nc" not in _NC_CACHE:
        _NC_CACHE["nc"] = _build_nc()
    return _NC_CACHE["nc"]


def kernel(x, weight, bias, _trace=False, _trace_kwargs=None):
    from concourse.bass_utils import run_bass_kernel_spmd

    x = np.asarray(x, dtype=np.float32)
    weight = np.asarray(weight, dtype=np.float32)
    bias = np.asarray(bias, dtype=np.float32)
    xp, wt, bv = _host_prep(x, weight, bias)

    nc = _get_nc()
    in_maps = [{"xp": xp[b], "wt": wt, "bv": bv} for b in range(B)]
    res = run_bass_kernel_spmd(
        nc, in_maps, list(range(NCORES)),
        trace=_trace, **(_trace_kwargs or {}),
    )
    out = np.empty((B, CO, NH, NW), dtype=np.float32)
    for b in range(B):
        ph = res.results[b]["out"].reshape(4, CO, NP, NP).astype(np.float32)
        for py in range(2):
            for px in range(2):
                out[b, :, py::2, px::2] = ph[py * 2 + px]
    if _trace:
        kernel._last_results = res
    return out


# revision 12
# speedup vs baseline: 1.1252x; 1.0074x over previous
"""Trainium2 Bass kernel for a stride-2 4x4 ConvTranspose2d with
per-kernel-position bias (bias added before the overlap-add fold).

Shapes (hardcoded):
  x:      (8, 256, 64, 64)  f32
  weight: (128, 256, 4, 4)  f32
  bias:   (128, 4, 4)       f32
  out:    (8, 128, 130, 130) f32   [nh = (64-1)*2+4 = 130]

Strategy: data-parallel over batch - one sample per NeuronCore, 8 cores.
Per core the deconv is computed as 4 output-phase planes (p%2, q%2), each
a 65x65 image. Each phase plane is the sum of 4 shifted matmuls (the
kernel positions sharing that parity) accumulated directly in PSUM:

  plane[o, P, Q] = sum_{a,b in {0,1}} W[:, :, py+2a, px+2b]^T @ xp[:, P-a+1, Q-b+1]

All matmul inputs are bf16 (rel err ~4e-3 of output scale, far under the
2e-2 gate); bf16 enables FWL so LDWEIGHTS fully overlaps streaming and
the PE runs at its ~2.4GHz roofline. Inputs are spread across four DMA
queues with explicit completion-ordering (add_dep_helper) so the first
chunk's data (xp rows 0-15 + phase-0 weights) is not round-robin-delayed
by the bulk. The per-kernel-position bias with edge corrections baked in
is 3 host-precomputed rows (top/interior/bottom) per phase; drains add
them via stride-0 broadcast APs, writing bf16 planes streamed out in 3
DMAs per phase. The host interleaves the 4 bf16 planes into the strided
f32 (130,130) output.
"""

import numpy as np

B, CI, H, W = 8, 256, 64, 64
CO, KH, KW = 128, 4, 4
NH = NW = 130
NP = 65          # phase plane side
PADH = 66        # padded x rows (+1 top, +1 bottom)
PADW = 68        # padded x cols (+1 left, +3 right; row stride 2*68 elems)
NCORES = 8

CHUNK_ROWS = [7, 7, 7, 7, 7, 7, 7, 7, 7, 2]
assert sum(CHUNK_ROWS) == NP
N_WARMUP = 10


def _build_nc():
    import concourse.mybir as mybir
    import concourse.tile as tile
    from concourse import bacc
    from concourse.tile_rust import add_dep_helper

    f32 = mybir.dt.float32
    bf16 = mybir.dt.bfloat16

    nc = bacc.Bacc(None)
    # xp layout: [i=128, row=66, kt=2, col=68] bf16 - kt inner so a row
    # range is one contiguous DMA descriptor.
    xp_ext = nc.declare_dram_parameter("xp", [128, PADH * 2 * PADW], bf16, isOutput=False)
    # wt layout: [i=128, phase=4, kt=2, tap=4, o=128] bf16 - phase outer so
    # phase-0 weights arrive in one small early DMA.
    wt_ext = nc.declare_dram_parameter("wt", [128, 4 * 2 * 4 * CO], bf16, isOutput=False)
    # bias rows: [o=128, phase=4, kind=3 (top/int/bottom), 65] f32
    bv_ext = nc.declare_dram_parameter("bv", [128, 4 * 3 * NP], f32, isOutput=False)
    out_ext = nc.declare_dram_parameter("out", [4, CO, NP * NP], bf16, isOutput=True)

    with tile.TileContext(nc) as tc:
        with (
            tc.tile_pool(name="const", bufs=1) as cpool,
            tc.tile_pool(name="psum", bufs=6, space="PSUM") as ppool,
        ):
            xp_t = cpool.tile([128, PADH, 2, PADW], bf16, tag="xp", name="xp")
            w_t = cpool.tile([128, 4, 2, 4, CO], bf16, tag="w", name="w")
            bv_t = cpool.tile([128, 4, 3, NP], f32, tag="bv", name="bv")
            planes = [
                cpool.tile([128, NP, NP], bf16, tag=f"plane{p}", name=f"plane{p}")
                for p in range(4)
            ]

            # Input DMAs on the three DMA-capable queues. Completion-chaining
            # via add_dep_helper(later, earlier) - the LATER DMA's queue
            # instruction carries a wait on the EARLIER one's completion
            # semaphore - so in-queue round-robin can't starve the pieces the
            # first chunks need (xp rows 0-24 + phase-0 weights):
            #   sync:   xp rows 0-24 (chunks 0-3), later the out DMAs
            #   gpsimd: xp rows 24-43 free-running; rows 43-66 chained after
            #   scalar: w phase0 -> bias rows -> w phases 1-3 (chained)
            g0 = nc.sync.dma_start(xp_t[:, 0:24], xp_ext[:, 0:24 * 2 * PADW])
            w0 = nc.scalar.dma_start(w_t[:, 0], wt_ext[:, 0:2 * 4 * CO])
            bv = nc.scalar.dma_start(bv_t[:], bv_ext[:])
            w123 = nc.scalar.dma_start(w_t[:, 1:4], wt_ext[:, 2 * 4 * CO:])
            mid = nc.gpsimd.dma_start(
                xp_t[:, 24:43], xp_ext[:, 24 * 2 * PADW:43 * 2 * PADW]
            )
            hi = nc.gpsimd.dma_start(
                xp_t[:, 43:66], xp_ext[:, 43 * 2 * PADW:]
            )
            add_dep_helper(mid.ins, g0.ins, sync=True, reason="xp-mid waits xp-g0")
            add_dep_helper(hi.ins, mid.ins, sync=True, reason="xp-hi waits xp-mid")
            add_dep_helper(bv.ins, w0.ins, sync=True, reason="bias waits w0")
            add_dep_helper(w123.ins, bv.ins, sync=True, reason="w123 waits bias")

            # PE warm-up: dummy bf16 matmuls with no input deps run during
            # the input-load window so HAM un-throttles (~5us of sustained
            # PE activity) by the time real matmuls begin. The memset goes on
            # the vector engine, which has no DMA queue and is never blocked.
            warm_in = cpool.tile([128, 512], bf16, tag="warm_in", name="warm_in")
            warm_ps = ppool.tile([128, 512], f32, tag="warm_ps", name="warm_ps", bufs=1)
            nc.vector.memset(warm_in[:], 1.0)
            for _ in range(N_WARMUP):
                nc.tensor.matmul(
                    warm_ps[:], warm_in[:, 0:128], warm_in[:],
                    start=True, stop=True,
                )

            for pidx in range(4):
                pstart = 0
                for ci, pn in enumerate(CHUNK_ROWS):
                    nn = pn * NP
                    ps = ppool.tile([128, 512], f32, tag="acc", name=f"acc{pidx}_{ci}")
                    mm = 0
                    for a in (0, 1):
                        for b in (0, 1):
                            for kt in (0, 1):
                                lhsT = w_t[:, pidx, kt, a * 2 + b, :]
                                rhs = xp_t[
                                    :,
                                    pstart + 1 - a: pstart + 1 - a + pn,
                                    kt,
                                    1 - b: 1 - b + NP,
                                ]
                                nc.tensor.matmul(
                                    ps[:, 0:nn], lhsT, rhs,
                                    start=(mm == 0), stop=(mm == 7),
                                )
                                mm += 1
                    # drain: PSUM + bias rows -> bf16 plane.
                    # bias row kinds: 0=top (P=0), 1=interior, 2=bottom (P=64)
                    pl = planes[pidx]
                    if ci == 0:
                        nc.vector.tensor_add(
                            pl[:, 0:1, :],
                            ps[:, 0:NP].rearrange("p (a b) -> p a b", b=NP),
                            bv_t[:, pidx, 0:1, :],
                        )
                        nc.vector.tensor_add(
                            pl[:, 1:7, :],
                            ps[:, NP:7 * NP].rearrange("p (a b) -> p a b", b=NP),
                            bv_t[:, pidx, 1:2, :].broadcast_to([128, 6, NP]),
                        )
                    elif ci == 9:
                        nc.vector.tensor_add(
                            pl[:, 63:64, :],
                            ps[:, 0:NP].rearrange("p (a b) -> p a b", b=NP),
                            bv_t[:, pidx, 1:2, :],
                        )
                        nc.vector.tensor_add(
                            pl[:, 64:65, :],
                            ps[:, NP:2 * NP].rearrange("p (a b) -> p a b", b=NP),
                            bv_t[:, pidx, 2:3, :],
                        )
                    else:
                        nc.vector.tensor_add(
                            pl[:, pstart:pstart + pn, :],
                            ps[:, 0:nn].rearrange("p (a b) -> p a b", b=NP),
                            bv_t[:, pidx, 1:2, :].broadcast_to([128, pn, NP]),
                        )
                    pstart += pn
                    if ci == 4:
                        nc.sync.dma_start(
                            out_ext[pidx, :, 0:35 * NP],
                            planes[pidx][:, 0:35, :],
                        )
                    elif ci == 7:
                        nc.sync.dma_start(
                            out_ext[pidx, :, 35 * NP:56 * NP],
                            planes[pidx][:, 35:56, :],
                        )
                    elif ci == 9:
                        nc.sync.dma_start(
                            out_ext[pidx, :, 56 * NP:],
                            planes[pidx][:, 56:, :],
                        )
    nc.compile()
    return nc


def _host_prep(x, weight, bias):
    import ml_dtypes

    # padded, i-tiled x: [B, 128, row=66, kt=2, col=68] bf16
    xp = np.zeros((B, 128, PADH, 2, PADW), dtype=ml_dtypes.bfloat16)
    xr = x.reshape(B, 2, 128, H, W).transpose(0, 2, 3, 1, 4)  # b,i,h,kt,w
    xp[:, :, 1:65, :, 1:65] = xr.astype(ml_dtypes.bfloat16)
    xp = np.ascontiguousarray(xp.reshape(B, 128, PADH * 2 * PADW))

    # weights as lhsT: wt[i, phase, kt, tap=(a,b), o] = weight[o, kt*128+i, py+2a, px+2b]
    wr = weight.reshape(CO, 2, 128, 4, 4)
    wt = np.empty((128, 4, 2, 4, CO), dtype=ml_dtypes.bfloat16)
    for py in range(2):
        for px in range(2):
            for a in range(2):
                for bb in range(2):
                    wt[:, py * 2 + px, :, a * 2 + bb, :] = (
                        wr[:, :, :, py + 2 * a, px + 2 * bb]
                        .transpose(2, 1, 0).astype(ml_dtypes.bfloat16)
                    )
    wt = np.ascontiguousarray(wt).reshape(128, 4 * 2 * 4 * CO)

    # bias rows [o, phase, kind=3, 65] f32: kind 0 = P=0 (top), 1 = interior
    # P, 2 = P=64 (bottom); per-column validity baked in.
    bv = np.zeros((128, 4, 3, NP), dtype=np.float32)
    bias = bias.astype(np.float32)
    for py in range(2):
        for px in range(2):
            p = py * 2 + px
            for kind, avalid in ((0, (0,)), (1, (0, 1)), (2, (1,))):
                for q in range(NP):
                    s = np.zeros(128, dtype=np.float32)
                    for a in range(2):
                        if a not in avalid:
                            continue
                        for b2 in range(2):
                            if q == 0 and b2 == 1:
                                continue
                            if q == NP - 1 and b2 == 0:
                                continue
                            s += bias[:, py + 2 * a, px + 2 * b2]
                    bv[:, p, kind, q] = s
    bv = np.ascontiguousarray(bv.reshape(128, 4 * 3 * NP))
    return xp, wt, bv


_NC_CACHE = {}


def _get_nc():
    if "nc" not in _NC_CACHE:
        _NC_CACHE["nc"] = _build_nc()
    return _NC_CACHE["nc"]


def kernel(x, weight, bias, _trace=False, _trace_kwargs=None):
    from concourse.bass_utils import run_bass_kernel_spmd

    x = np.asarray(x, dtype=np.float32)
    weight = np.asarray(weight, dtype=np.float32)
    bias = np.asarray(bias, dtype=np.float32)
    xp, wt, bv = _host_prep(x, weight, bias)

    nc = _get_nc()
    in_maps = [{"xp": xp[b], "wt": wt, "bv": bv} for b in range(B)]
    res = run_bass_kernel_spmd(
        nc, in_maps, list(range(NCORES)),
        trace=_trace, **(_trace_kwargs or {}),
    )
    out = np.empty((B, CO, NH, NW), dtype=np.float32)
    for b in range(B):
        ph = res.results[b]["out"].reshape(4, CO, NP, NP).astype(np.float32)
        for py in range(2):
            for px in range(2):
                out[b, :, py::2, px::2] = ph[py * 2 + px]
    if _trace:
        kernel._last_results = res
    return out
